# revision 19
# baseline (speedup 1.0000x reference)
"""Multi-head attention (B=4, S=2048, D=1024, H=16) on 8 trn2 NeuronCores.

Sharding: (batch, head-half) -> one core each. Core c handles batch c//2 and
heads (c%2)*8 .. (c%2)*8+7 (feature columns (c%2)*512 .. +512 of the QKV
projections, rows of Wo). Each core computes its 8 heads' attention and a
partial output projection; the host sums the two partials per batch and adds
the output bias.

Device layout per core (S=2048 tokens, F=512 local features, hd=64):
  - inputs Q/K/V arrive host-transposed as [1024, 2048] so the d_model
    contraction sits on SBUF partitions,
  - q^T/k^T are produced feature-major ([512, 2048]) via lhsT=W, rhs=X^T,
  - scores are computed transposed (S^T[k, q]) so the P@V matmul can use v
    in natural [token, feature] layout as the stationary operand,
  - softmax: exp on the ACT engine with the 1/8 scale folded in; the
    denominator comes from an all-ones 65th column appended to v; one
    [33,512] reciprocal per group (denominator rows packed at partitions
    0/32) is PE-broadcast for the normalizing multiply.

Schedule: the ACT engine's exp stream (256 x N=1024 activates ~ 285us) is
the pacing resource, so it is started as early as possible: the serial head
is only K-projection f-tile 0 (all four token chunks) + Q chunk-0 f-tile 0,
after which group (0,0)'s scores/exp stream begins (~8us). V pieces and the
remaining K/Q f-tiles are emitted before the group that consumes them
(keeping the shared PSUM accumulator ring acyclic) and execute inside the
exp stream's PE slack. V arrives via eight [128,1024] row-block DMAs per
column wave (2KB/partition descriptors). The partial output is stored bf16;
the host sums partials in fp32.
"""
import numpy as np

import concourse.bass as bass
import concourse.tile as tile
from concourse import mybir
from concourse.bass_utils import run_bass_kernel_spmd

F32 = mybir.dt.float32
F32R = mybir.dt.float32r
BF16 = mybir.dt.bfloat16
EXP = mybir.ActivationFunctionType.Exp

B, S, DM, H_TOT = 4, 2048, 1024, 16
F = 512          # features per core (8 heads x 64)
HD = 64          # head dim
NH = 8           # heads per core
NP = 4           # head pairs per core
KT = 16          # k tiles of 128
NQT = 4          # q chunks of 512
SCALE = 0.125    # 1/sqrt(64)
N_CORES = 8

_WAIT_CAP = {"InstEventSemaphore": 2}


def _split_multiwaits(nc):
    """This walrus build accepts 1 sync-wait per instruction (2 on
    EventSemaphore); spread extras over same-engine NOPs placed before."""
    n_spill = 0
    for f in nc.m.functions:
        for bb in f.blocks:
            new = []
            changed = False
            for inst in bb.instructions:
                si = inst.sync_info
                cap = _WAIT_CAP.get(type(inst).__name__, 1)
                if si is not None and len(si.on_wait) > cap:
                    extra = list(si.on_wait[: len(si.on_wait) - cap])
                    del si.on_wait[: len(si.on_wait) - cap]
                    for w in extra:
                        n_spill += 1
                        nop = mybir.InstNoOp(name=f"I-wspill-{n_spill}-{inst.name}")
                        nop.engine = inst.engine
                        nop.sync_info = mybir.SyncInfo(on_wait=[w], on_update=[])
                        new.append(nop)
                    changed = True
                new.append(inst)
            if changed:
                bb.instructions[:] = new
    return n_spill


def build_program():
    nc = bass.Bass("TRN2", target_bir_lowering=False, debug=False, num_devices=1)

    d_qt = nc.dram_tensor("qt", [DM, S], BF16, kind="ExternalInput").ap()
    d_kt = nc.dram_tensor("kt", [DM, S], BF16, kind="ExternalInput").ap()
    d_vt = nc.dram_tensor("vt", [DM, S], BF16, kind="ExternalInput").ap()
    d_wq = nc.dram_tensor("wq", [DM, F], BF16, kind="ExternalInput").ap()
    d_wk = nc.dram_tensor("wk", [DM, F], BF16, kind="ExternalInput").ap()
    d_wv = nc.dram_tensor("wv", [DM, F], BF16, kind="ExternalInput").ap()
    d_wo = nc.dram_tensor("wo", [F, DM], BF16, kind="ExternalInput").ap()
    d_bq = nc.dram_tensor("bq", [F], F32, kind="ExternalInput").ap()
    d_bk = nc.dram_tensor("bk", [F], F32, kind="ExternalInput").ap()
    d_bv = nc.dram_tensor("bv", [F], F32R, kind="ExternalInput").ap()
    d_ones = nc.dram_tensor("ones", [1, 128], F32R, kind="ExternalInput").ap()
    d_part = nc.dram_tensor("part", [S, DM], BF16, kind="ExternalOutput").ap()

    with tile.TileContext(nc) as tc:
        with (
            tc.tile_pool(name="wpool", bufs=1) as wpool,
            tc.tile_pool(name="big", bufs=1) as big,
            tc.tile_pool(name="kinch", bufs=32) as kinch,
            tc.tile_pool(name="qinch", bufs=16) as qinch,
            tc.tile_pool(name="vin", bufs=1) as vin,
            tc.tile_pool(name="exch", bufs=10) as exch,
            tc.tile_pool(name="small", bufs=2) as small,
            tc.tile_pool(name="outst", bufs=2) as outst,
            tc.tile_pool(name="rcp", bufs=1) as rcp,
            tc.tile_pool(name="ocp", bufs=3) as ocp,
            tc.tile_pool(name="ps_sc", bufs=2, space="PSUM") as ps_sc,
            tc.tile_pool(name="ps_pv", bufs=2, space="PSUM") as ps_pv,
            tc.tile_pool(name="ps_fill", bufs=1, space="PSUM") as ps_fill,
            tc.tile_pool(name="ps_misc", bufs=1, space="PSUM") as ps_misc,
        ):
            # ---- resident tiles
            wq_sb = [wpool.tile([128, F], BF16, tag=f"wq{m}", name=f"wq{m}") for m in range(8)]
            wk_sb = [wpool.tile([128, F], BF16, tag=f"wk{m}", name=f"wk{m}") for m in range(8)]
            wv_sb = [wpool.tile([128, F], BF16, tag=f"wv{m}", name=f"wv{m}") for m in range(8)]
            wo_sb = [wpool.tile([128, DM], BF16, tag=f"wo{f}", name=f"wo{f}") for f in range(4)]
            qT_sb = [big.tile([128, S], BF16, tag=f"qT{f}", name=f"qT{f}") for f in range(4)]
            kT_sb = [big.tile([128, S], BF16, tag=f"kT{f}", name=f"kT{f}") for f in range(4)]
            oT_sb = [big.tile([128, S], BF16, tag=f"oT{f}", name=f"oT{f}") for f in range(4)]
            v_sb = [big.tile([128, NH * (HD + 1)], BF16, tag=f"v{t}", name=f"v{t}") for t in range(KT)]
            bq_sb = wpool.tile([128, 4], F32, tag="bq")
            bk_sb = wpool.tile([128, 4], F32, tag="bk")
            bv_sb = wpool.tile([1, F], F32R, tag="bv")
            ones_sb = wpool.tile([1, 128], F32R, tag="ones")
            bvbc_sb = wpool.tile([128, F], F32, tag="bvbc")

            # ---- DMA issue order follows consumption order: K inputs for
            # f-tile 0 first (starts the exp stream), then Q chunk 0, then
            # the rest of K, then V row-blocks, then Wo.
            nc.sync.dma_start(ones_sb[:], d_ones[:])
            nc.sync.dma_start(bk_sb[:], d_bk.rearrange("(f p) -> p f", p=128))
            for m in range(8):
                nc.sync.dma_start(wk_sb[m][:], d_wk[128 * m:128 * (m + 1), :])

            def load_chunk(src, n, pool):
                chs = []
                for m in range(8):
                    ch = pool.tile([128, 512], BF16, tag="inch")
                    nc.sync.dma_start(
                        ch[:],
                        src[128 * m:128 * (m + 1), 512 * n:512 * (n + 1)],
                    )
                    chs.append(ch)
                return chs

            kchs = [load_chunk(d_kt, 0, kinch)]
            nc.sync.dma_start(bq_sb[:], d_bq.rearrange("(f p) -> p f", p=128))
            for m in range(8):
                nc.sync.dma_start(wq_sb[m][:], d_wq[128 * m:128 * (m + 1), :])
            qchs0 = load_chunk(d_qt, 0, qinch)
            for n in range(1, NQT):
                kchs.append(load_chunk(d_kt, n, kinch))
            for m in range(8):
                nc.sync.dma_start(wv_sb[m][:], d_wv[128 * m:128 * (m + 1), :])
            nc.sync.dma_start(bv_sb[:], d_bv.rearrange("(a f) -> a f", a=1))

            vblk = None

            def v_wave(w):
                nonlocal vblk
                blocks = []
                for m in range(8):
                    vb = vin.tile([128, 1024], BF16, tag=f"vb{m}", name=f"vb{m}w{w}")
                    nc.sync.dma_start(
                        vb[:], d_vt[128 * m:128 * (m + 1), 1024 * w:1024 * (w + 1)]
                    )
                    blocks.append(vb)
                vblk = blocks

            v_wave(0)
            for f in range(4):
                nc.sync.dma_start(wo_sb[f][:], d_wo[128 * f:128 * (f + 1), :])

            # ---- PE warmup spinner: ~4.5us of tiny matmuls so the HAM
            # clock gate reaches 8/8 while the initial DMAs land.
            warm = ps_misc.tile([64, 128], F32, tag="ps", name="warm")
            for _ in range(56):
                nc.tensor.matmul(warm[:], ones_sb[0:1, 0:64], ones_sb[0:1, :])
            warm_out = small.tile([64, 128], BF16, tag="wm", name="warm_out")
            with nc.allow_low_precision(reason="warmup drain"):
                nc.vector.tensor_copy(warm_out[:], warm[:])

            # bv broadcast over partitions via K=1 matmul
            psbv = ps_misc.tile([128, 512], F32, tag="ps", name="psbv")
            nc.tensor.matmul(psbv[:], ones_sb[0:1, :], bv_sb[0:1, :])
            nc.vector.tensor_copy(bvbc_sb[:], psbv[:])

            def proj_piece(w_sb, chs, bias_sb, dst_sb, n, f):
                # one feature tile: 8 K-contiguous accumulating matmuls
                accp = ps_fill.tile([128, 512], F32, tag="fl", name="accp")
                for m in range(8):
                    nc.tensor.matmul(
                        accp[:],
                        w_sb[m][:, 128 * f:128 * (f + 1)],
                        chs[m][:],
                        start=(m == 0),
                        stop=(m == 7),
                    )
                with nc.allow_low_precision(reason="bf16 qT/kT store"):
                    nc.vector.tensor_scalar_add(
                        dst_sb[f][:, 512 * n:512 * (n + 1)],
                        accp[:],
                        bias_sb[:, f:f + 1],
                    )

            # ---- v projection (natural layout, bf16, ones column per head)
            def v_piece(t):
                acc = ps_fill.tile([128, 512], F32, tag="fl", name="accv")
                tof = 128 * (t % 8)
                for m in range(8):
                    nc.tensor.matmul(
                        acc[:], vblk[m][:, tof:tof + 128], wv_sb[m][:],
                        start=(m == 0), stop=(m == 7)
                    )
                v3 = v_sb[t][:].rearrange("p (h e) -> p h e", e=HD + 1)
                nc.vector.memset(v3[:, :, HD:HD + 1], 1.0)
                nc.vector.tensor_add(
                    v3[:, :, 0:HD],
                    acc[:].rearrange("p (h e) -> p h e", e=HD),
                    bvbc_sb[:].rearrange("p (h e) -> p h e", e=HD),
                )

            # ---- serial head: K f-tile 0 over all chunks + Q chunk-0
            # f-tile 0 (what the first scores need), then all V pieces
            # (what group (0,0)'s PV needs -- these run in the exp
            # stream's PE slack as the V row-blocks land).
            for c in range(NQT):
                proj_piece(wk_sb, kchs[c], bk_sb, kT_sb, c, 0)
            proj_piece(wq_sb, qchs0, bq_sb, qT_sb, 0, 0)
            v_piece(0)

            # ---- attention + output projection
            wo_pending = []

            def emit_wo(count):
                for _ in range(count):
                    if not wo_pending:
                        return
                    tt, j = wo_pending.pop(0)
                    tsl = slice(128 * tt, 128 * (tt + 1))
                    pw = ps_misc.tile([128, 512], F32, tag="ps", name="pw")
                    for f in range(4):
                        nc.tensor.matmul(
                            pw[:], oT_sb[f][:, tsl],
                            wo_sb[f][:, 512 * j:512 * (j + 1)],
                            start=(f == 0), stop=(f == 3),
                        )
                    ost = outst.tile([128, 512], BF16, tag="outst")
                    with nc.allow_low_precision(reason="bf16 partial out"):
                        nc.vector.tensor_copy(ost[:], pw[:])
                    nc.sync.dma_start(
                        d_part[tsl, 512 * j:512 * (j + 1)], ost[:]
                    )

            def norm_phase1(po):
                # evacuate o' from PSUM right away so the accumulator bank
                # frees for the next group's PV
                oc = ocp.tile([65, 512], F32, tag="oc", name="oc")
                nc.vector.tensor_copy(oc[:], po[0:65, :])
                return oc

            dn2_t = small.tile([33, 512], F32, tag="dn2", name="dn2t", bufs=1)
            nc.vector.memset(dn2_t[:], 1.0)
            rc_t = rcp.tile([33, 512], F32, tag="rc", name="rct", bufs=1)

            def recip_pair(ocA, ocB):
                # both denominator rows packed at partitions 0/32 of one
                # tile; a single reciprocal (cost tracks free-dim) serves
                # both halves
                nc.vector.tensor_copy(dn2_t[0:1, :], ocA[64:65, :])
                nc.vector.tensor_copy(dn2_t[32:33, :], ocB[64:65, :])
                nc.vector.reciprocal(rc_t[:], dn2_t[:])
                dnrA = small.tile([1, 512], F32R, tag="dnrA", name="dnrA", bufs=2)
                nc.vector.tensor_copy(dnrA[0:1, :], rc_t[0:1, :])
                dnrB = small.tile([1, 512], F32R, tag="dnrB", name="dnrB", bufs=2)
                nc.vector.tensor_copy(dnrB[0:1, :], rc_t[32:33, :])
                return (dnrA, dnrB)

            def make_norm(p, qsl, i, oc, dnr):
                # PE-broadcast the reciprocal row, multiply o'^T from PSUM
                def norm():
                    r0 = 64 * i
                    pb = ps_misc.tile([128, 512], F32, tag="ps", name="pb")
                    nc.tensor.matmul(pb[0:64, :], ones_sb[0:1, 0:64],
                                     dnr[i][0:1, :])
                    with nc.allow_low_precision(reason="bf16 normalized out"):
                        nc.vector.tensor_mul(
                            oT_sb[p][r0:r0 + 64, qsl], oc[0:64, :], pb[0:64, :]
                        )
                return norm

            pending_norms = []
            pending_norms2 = []

            for n in range(NQT):
                if n + 1 < NQT:
                    next_qchs = load_chunk(d_qt, n + 1, qinch)
                qsl = slice(512 * n, 512 * (n + 1))
                for p in range(NP):
                    # per-slot fillers: group (0,0) consumes one v piece per
                    # k-tile (emitted just ahead of the PV that needs it);
                    # groups (0,p) consume the next pair's k/q f-tiles.
                    fillers = []
                    if n == 0:
                        if p == 0:
                            fillers = [(('v', t), None) for t in range(1, KT)]
                        if p < NP - 1:
                            fillers += [(('k', c, p + 1), None) for c in range(NQT)]
                            fillers.append((('q', p + 1), None))

                    def consume_filler():
                        if not fillers:
                            return
                        key, _ = fillers.pop(0)
                        if key[0] == 'v':
                            t = key[1]
                            if t == 8:
                                v_wave(1)
                            v_piece(t)
                        elif key[0] == 'k':
                            proj_piece(wk_sb, kchs[key[1]], bk_sb, kT_sb,
                                       key[1], key[2])
                        else:
                            proj_piece(wq_sb, qchs0, bq_sb, qT_sb, 0, key[1])

                    poA = ps_pv.tile([128, 512], F32, tag="po")
                    poB = ps_pv.tile([128, 512], F32, tag="po")

                    def sc_emit(m, p=p, qsl=qsl):
                        scp = ps_sc.tile([128, 1024], F32, tag="sc")
                        ksl = slice(128 * m, 128 * (m + 1))
                        nc.tensor.matmul(
                            scp[:, 0:512], kT_sb[p][0:64, ksl], qT_sb[p][0:64, qsl],
                            tile_position=(0, 0),
                        )
                        nc.tensor.matmul(
                            scp[:, 512:1024], kT_sb[p][64:128, ksl],
                            qT_sb[p][64:128, qsl], tile_position=(64, 0),
                        )
                        ex = exch.tile([128, 1024], BF16, tag="ex")
                        nc.scalar.activation(ex[:], scp[:], EXP, scale=SCALE)
                        return ex

                    exs = {0: sc_emit(0), 1: sc_emit(1)}
                    for m in range(KT):
                        # previous group's evacuation first so its PSUM
                        # banks free before this group's PV needs them
                        if m == 0 and len(pending_norms) >= 2:
                            poA_, pA_, qslA_, iA_ = pending_norms.pop(0)
                            poB_, pB_, qslB_, iB_ = pending_norms.pop(0)
                            ocA = norm_phase1(poA_)
                            ocB = norm_phase1(poB_)
                            dnr = recip_pair(ocA, ocB)
                            pending_norms2.append(
                                make_norm(pA_, qslA_, iA_, ocA, dnr))
                            pending_norms2.append(
                                make_norm(pB_, qslB_, iB_, ocB, dnr))
                        if m + 2 < KT:
                            exs[m + 2] = sc_emit(m + 2)
                        ex = exs.pop(m)
                        nc.tensor.matmul(
                            poA[0:65, :], v_sb[m][:, 130 * p:130 * p + 65],
                            ex[:, 0:512], start=(m == 0), stop=(m == KT - 1),
                        )
                        nc.tensor.matmul(
                            poB[0:65, :], v_sb[m][:, 130 * p + 65:130 * p + 130],
                            ex[:, 512:1024], start=(m == 0), stop=(m == KT - 1),
                        )
                        if m in (4, 6) and pending_norms2:
                            pending_norms2.pop(0)()
                        if m in (7, 10, 13):
                            emit_wo(1)
                        consume_filler()
                        if m >= 6 and len(fillers) > KT - m:
                            consume_filler()
                    while fillers:
                        consume_filler()
                    if n + 1 < NQT:
                        # next chunk's q projection, one feature tile per group
                        proj_piece(wq_sb, next_qchs, bq_sb, qT_sb, n + 1, p)
                    pending_norms.append((poA, p, qsl, 0))
                    pending_norms.append((poB, p, qsl, 1))
                    if n == NQT - 1:
                        # final chunk: run norms eagerly so the tail is short
                        while pending_norms2:
                            pending_norms2.pop(0)()
                        while pending_norms:
                            poA_, pA_, qslA_, iA_ = pending_norms.pop(0)
                            poB_, pB_, qslB_, iB_ = pending_norms.pop(0)
                            ocA = norm_phase1(poA_)
                            ocB = norm_phase1(poB_)
                            dnr = recip_pair(ocA, ocB)
                            make_norm(pA_, qslA_, iA_, ocA, dnr)()
                            make_norm(pB_, qslB_, iB_, ocB, dnr)()
                        emit_wo(2)
                # queue this chunk's Wo pieces (interleaved into later groups)
                for t in range(4):
                    for j in range(2):
                        wo_pending.append((4 * n + t, j))
            while pending_norms:
                poA_, pA_, qslA_, iA_ = pending_norms.pop(0)
                poB_, pB_, qslB_, iB_ = pending_norms.pop(0)
                ocA = norm_phase1(poA_)
                ocB = norm_phase1(poB_)
                dnr = recip_pair(ocA, ocB)
                make_norm(pA_, qslA_, iA_, ocA, dnr)()
                make_norm(pB_, qslB_, iB_, ocB, dnr)()
            for nm in pending_norms2:
                nm()
            emit_wo(len(wo_pending))

    _split_multiwaits(nc)
    return nc


_PROGRAM = None


def _get_program():
    global _PROGRAM
    if _PROGRAM is None:
        _PROGRAM = build_program()
    return _PROGRAM


def make_in_maps(Q, K, V, Wq, bq, Wk, bk, Wv, bv, Wo, bo):
    import ml_dtypes
    bf = lambda x: np.asarray(x, dtype=np.float32).astype(ml_dtypes.bfloat16)
    f32 = lambda x: np.asarray(x, dtype=np.float32)
    Q, K, V = bf(Q), bf(K), bf(V)
    Wq, Wk, Wv, Wo = bf(Wq), bf(Wk), bf(Wv), bf(Wo)
    bq, bk, bv = f32(bq), f32(bk), f32(bv)
    ones = np.ones((1, 128), np.float32)
    in_maps = []
    for c in range(N_CORES):
        b, hh = c // 2, c % 2
        fs = slice(F * hh, F * (hh + 1))
        in_maps.append({
            "qt": np.ascontiguousarray(Q[b].T),
            "kt": np.ascontiguousarray(K[b].T),
            "vt": np.ascontiguousarray(V[b].T),
            "wq": np.ascontiguousarray(Wq[:, fs]),
            "wk": np.ascontiguousarray(Wk[:, fs]),
            "wv": np.ascontiguousarray(Wv[:, fs]),
            "wo": np.ascontiguousarray(Wo[fs, :]),
            "bq": np.ascontiguousarray(bq[fs]),
            "bk": np.ascontiguousarray(bk[fs]),
            "bv": np.ascontiguousarray(bv[fs]),
            "ones": ones,
        })
    return in_maps


def kernel(Q, K, V, Wq, bq, Wk, bk, Wv, bv, Wo, bo, _trace=False, _trace_kwargs=None):
    nc = _get_program()
    in_maps = make_in_maps(Q, K, V, Wq, bq, Wk, bk, Wv, bv, Wo, bo)
    res = run_bass_kernel_spmd(
        nc, in_maps, core_ids=list(range(N_CORES)),
        trace=_trace, **(_trace_kwargs or {}),
    )
    parts = [np.asarray(r["part"], dtype=np.float32) for r in res.results]
    out = np.stack([parts[2 * b] + parts[2 * b + 1] for b in range(B)])
    out += np.asarray(bo, dtype=np.float32)[None, None, :]
    if _trace:
        return out, res
    return out


# revision 20
# speedup vs baseline: 1.0210x; 1.0210x over previous
"""Multi-head attention (B=4, S=2048, D=1024, H=16) on 8 trn2 NeuronCores.

Sharding: (batch, head-half) -> one core each. Core c handles batch c//2 and
heads (c%2)*8 .. (c%2)*8+7 (feature columns (c%2)*512 .. +512 of the QKV
projections, rows of Wo). Each core computes its 8 heads' attention and a
partial output projection; the host sums the two partials per batch and adds
the output bias.

Device layout per core (S=2048 tokens, F=512 local features, hd=64):
  - inputs Q/K/V arrive host-transposed as [1024, 2048] so the d_model
    contraction sits on SBUF partitions,
  - q^T/k^T are produced feature-major ([512, 2048]) via lhsT=W, rhs=X^T,
  - scores are computed transposed (S^T[k, q]) so the P@V matmul can use v
    in natural [token, feature] layout as the stationary operand,
  - softmax: exp on the ACT engine with the 1/8 scale folded in; the
    denominator comes from an all-ones 65th column appended to v; one
    [33,512] reciprocal per group (denominator rows packed at partitions
    0/32) is PE-broadcast for the normalizing multiply.

Schedule: the ACT engine's exp stream (256 x N=1024 activates ~ 285us) is
the pacing resource, so it is started as early as possible: the serial head
is only K-projection f-tile 0 (all four token chunks) + Q chunk-0 f-tile 0,
after which group (0,0)'s scores/exp stream begins (~8us). V pieces and the
remaining K/Q f-tiles are emitted before the group that consumes them
(keeping the shared PSUM accumulator ring acyclic) and execute inside the
exp stream's PE slack. V arrives via eight [128,1024] row-block DMAs per
column wave (2KB/partition descriptors). The partial output is stored bf16;
the host sums partials in fp32.
"""
import numpy as np

import concourse.bass as bass
import concourse.tile as tile
from concourse import mybir
from concourse.bass_utils import run_bass_kernel_spmd

F32 = mybir.dt.float32
F32R = mybir.dt.float32r
BF16 = mybir.dt.bfloat16
EXP = mybir.ActivationFunctionType.Exp

B, S, DM, H_TOT = 4, 2048, 1024, 16
F = 512          # features per core (8 heads x 64)
HD = 64          # head dim
NH = 8           # heads per core
NP = 4           # head pairs per core
KT = 16          # k tiles of 128
NQT = 4          # q chunks of 512
SCALE = 0.125    # 1/sqrt(64)
N_CORES = 8

_WAIT_CAP = {"InstEventSemaphore": 2}


def _split_multiwaits(nc):
    """This walrus build accepts 1 sync-wait per instruction (2 on
    EventSemaphore); spread extras over same-engine NOPs placed before."""
    n_spill = 0
    for f in nc.m.functions:
        for bb in f.blocks:
            new = []
            changed = False
            for inst in bb.instructions:
                si = inst.sync_info
                cap = _WAIT_CAP.get(type(inst).__name__, 1)
                if si is not None and len(si.on_wait) > cap:
                    extra = list(si.on_wait[: len(si.on_wait) - cap])
                    del si.on_wait[: len(si.on_wait) - cap]
                    for w in extra:
                        n_spill += 1
                        nop = mybir.InstNoOp(name=f"I-wspill-{n_spill}-{inst.name}")
                        nop.engine = inst.engine
                        nop.sync_info = mybir.SyncInfo(on_wait=[w], on_update=[])
                        new.append(nop)
                    changed = True
                new.append(inst)
            if changed:
                bb.instructions[:] = new
    return n_spill


def build_program():
    nc = bass.Bass("TRN2", target_bir_lowering=False, debug=False, num_devices=1)

    d_qt = nc.dram_tensor("qt", [DM, S], BF16, kind="ExternalInput").ap()
    d_kt = nc.dram_tensor("kt", [DM, S], BF16, kind="ExternalInput").ap()
    d_vt = nc.dram_tensor("vt", [DM, S], BF16, kind="ExternalInput").ap()
    d_wq = nc.dram_tensor("wq", [DM, F], BF16, kind="ExternalInput").ap()
    d_wk = nc.dram_tensor("wk", [DM, F], BF16, kind="ExternalInput").ap()
    d_wv = nc.dram_tensor("wv", [DM, F], BF16, kind="ExternalInput").ap()
    d_wo = nc.dram_tensor("wo", [F, DM], BF16, kind="ExternalInput").ap()
    d_bq = nc.dram_tensor("bq", [F], F32, kind="ExternalInput").ap()
    d_bk = nc.dram_tensor("bk", [F], F32, kind="ExternalInput").ap()
    d_bv = nc.dram_tensor("bv", [F], F32R, kind="ExternalInput").ap()
    d_ones = nc.dram_tensor("ones", [1, 128], F32R, kind="ExternalInput").ap()
    d_part = nc.dram_tensor("part", [S, DM], BF16, kind="ExternalOutput").ap()

    with tile.TileContext(nc) as tc:
        with (
            tc.tile_pool(name="wpool", bufs=1) as wpool,
            tc.tile_pool(name="big", bufs=1) as big,
            tc.tile_pool(name="kinch", bufs=32) as kinch,
            tc.tile_pool(name="qinch", bufs=16) as qinch,
            tc.tile_pool(name="vin", bufs=1) as vin,
            tc.tile_pool(name="exch", bufs=10) as exch,
            tc.tile_pool(name="small", bufs=2) as small,
            tc.tile_pool(name="outst", bufs=2) as outst,
            tc.tile_pool(name="rcp", bufs=1) as rcp,
            tc.tile_pool(name="ocp", bufs=3) as ocp,
            tc.tile_pool(name="ps_sc", bufs=2, space="PSUM") as ps_sc,
            tc.tile_pool(name="ps_pv", bufs=3, space="PSUM") as ps_pv,
            tc.tile_pool(name="ps_misc", bufs=1, space="PSUM") as ps_misc,
        ):
            # ---- resident tiles
            wq_sb = [wpool.tile([128, F], BF16, tag=f"wq{m}", name=f"wq{m}") for m in range(8)]
            wk_sb = [wpool.tile([128, F], BF16, tag=f"wk{m}", name=f"wk{m}") for m in range(8)]
            wv_sb = [wpool.tile([128, F], BF16, tag=f"wv{m}", name=f"wv{m}") for m in range(8)]
            wo_sb = [wpool.tile([128, DM], BF16, tag=f"wo{f}", name=f"wo{f}") for f in range(4)]
            qT_sb = [big.tile([128, S], BF16, tag=f"qT{f}", name=f"qT{f}") for f in range(4)]
            kT_sb = [big.tile([128, S], BF16, tag=f"kT{f}", name=f"kT{f}") for f in range(4)]
            oT_sb = [big.tile([128, S], BF16, tag=f"oT{f}", name=f"oT{f}") for f in range(4)]
            v_sb = [big.tile([128, NH * (HD + 1)], BF16, tag=f"v{t}", name=f"v{t}") for t in range(KT)]
            bq_sb = wpool.tile([128, 4], F32, tag="bq")
            bk_sb = wpool.tile([128, 4], F32, tag="bk")
            bv_sb = wpool.tile([1, F], F32R, tag="bv")
            ones_sb = wpool.tile([1, 128], F32R, tag="ones")
            bvbc_sb = wpool.tile([128, F], F32, tag="bvbc")

            # ---- DMA issue order follows consumption order: K inputs for
            # f-tile 0 first (starts the exp stream), then Q chunk 0, then
            # the rest of K, then V row-blocks, then Wo.
            nc.sync.dma_start(ones_sb[:], d_ones[:])
            nc.sync.dma_start(bk_sb[:], d_bk.rearrange("(f p) -> p f", p=128))
            for m in range(8):
                nc.sync.dma_start(wk_sb[m][:], d_wk[128 * m:128 * (m + 1), :])

            def load_chunk(src, n, pool):
                chs = []
                for m in range(8):
                    ch = pool.tile([128, 512], BF16, tag="inch")
                    nc.sync.dma_start(
                        ch[:],
                        src[128 * m:128 * (m + 1), 512 * n:512 * (n + 1)],
                    )
                    chs.append(ch)
                return chs

            kchs = [load_chunk(d_kt, 0, kinch)]
            nc.sync.dma_start(bq_sb[:], d_bq.rearrange("(f p) -> p f", p=128))
            for m in range(8):
                nc.sync.dma_start(wq_sb[m][:], d_wq[128 * m:128 * (m + 1), :])
            qchs0 = load_chunk(d_qt, 0, qinch)
            for n in range(1, NQT):
                kchs.append(load_chunk(d_kt, n, kinch))
            for m in range(8):
                nc.sync.dma_start(wv_sb[m][:], d_wv[128 * m:128 * (m + 1), :])
            nc.sync.dma_start(bv_sb[:], d_bv.rearrange("(a f) -> a f", a=1))

            vblk = None

            def v_wave(w):
                nonlocal vblk
                blocks = []
                for m in range(8):
                    vb = vin.tile([128, 1024], BF16, tag=f"vb{m}", name=f"vb{m}w{w}")
                    nc.sync.dma_start(
                        vb[:], d_vt[128 * m:128 * (m + 1), 1024 * w:1024 * (w + 1)]
                    )
                    blocks.append(vb)
                vblk = blocks

            v_wave(0)
            for f in range(4):
                nc.sync.dma_start(wo_sb[f][:], d_wo[128 * f:128 * (f + 1), :])

            # ---- PE warmup spinner: ~4.5us of tiny matmuls so the HAM
            # clock gate reaches 8/8 while the initial DMAs land.
            warm = ps_misc.tile([64, 128], F32, tag="ps", name="warm")
            for _ in range(56):
                nc.tensor.matmul(warm[:], ones_sb[0:1, 0:64], ones_sb[0:1, :])
            warm_out = small.tile([64, 128], BF16, tag="wm", name="warm_out")
            with nc.allow_low_precision(reason="warmup drain"):
                nc.vector.tensor_copy(warm_out[:], warm[:])

            # bv broadcast over partitions via K=1 matmul
            psbv = ps_misc.tile([128, 512], F32, tag="ps", name="psbv")
            nc.tensor.matmul(psbv[:], ones_sb[0:1, :], bv_sb[0:1, :])
            nc.vector.tensor_copy(bvbc_sb[:], psbv[:])

            def proj_piece(w_sb, chs, bias_sb, dst_sb, n, f):
                # one feature tile: 8 K-contiguous accumulating matmuls
                accp = ps_pv.tile([128, 512], F32, tag="po", name="accp")
                for m in range(8):
                    nc.tensor.matmul(
                        accp[:],
                        w_sb[m][:, 128 * f:128 * (f + 1)],
                        chs[m][:],
                        start=(m == 0),
                        stop=(m == 7),
                    )
                with nc.allow_low_precision(reason="bf16 qT/kT store"):
                    nc.vector.tensor_scalar_add(
                        dst_sb[f][:, 512 * n:512 * (n + 1)],
                        accp[:],
                        bias_sb[:, f:f + 1],
                    )

            # ---- v projection (natural layout, bf16, ones column per head)
            def v_piece(t):
                acc = ps_pv.tile([128, 512], F32, tag="po", name="accv")
                tof = 128 * (t % 8)
                for m in range(8):
                    nc.tensor.matmul(
                        acc[:], vblk[m][:, tof:tof + 128], wv_sb[m][:],
                        start=(m == 0), stop=(m == 7)
                    )
                v3 = v_sb[t][:].rearrange("p (h e) -> p h e", e=HD + 1)
                nc.vector.memset(v3[:, :, HD:HD + 1], 1.0)
                nc.vector.tensor_add(
                    v3[:, :, 0:HD],
                    acc[:].rearrange("p (h e) -> p h e", e=HD),
                    bvbc_sb[:].rearrange("p (h e) -> p h e", e=HD),
                )

            # ---- serial head: K f-tile 0 over all chunks + Q chunk-0
            # f-tile 0 (what the first scores need), then all V pieces
            # (what group (0,0)'s PV needs -- these run in the exp
            # stream's PE slack as the V row-blocks land).
            for c in range(NQT):
                proj_piece(wk_sb, kchs[c], bk_sb, kT_sb, c, 0)
            proj_piece(wq_sb, qchs0, bq_sb, qT_sb, 0, 0)
            for t in range(8):
                v_piece(t)
            v_wave(1)
            for t in range(8, KT):
                v_piece(t)

            # ---- attention + output projection
            wo_pending = []

            def emit_wo(count):
                for _ in range(count):
                    if not wo_pending:
                        return
                    tt, j = wo_pending.pop(0)
                    tsl = slice(128 * tt, 128 * (tt + 1))
                    pw = ps_misc.tile([128, 512], F32, tag="ps", name="pw")
                    for f in range(4):
                        nc.tensor.matmul(
                            pw[:], oT_sb[f][:, tsl],
                            wo_sb[f][:, 512 * j:512 * (j + 1)],
                            start=(f == 0), stop=(f == 3),
                        )
                    ost = outst.tile([128, 512], BF16, tag="outst")
                    with nc.allow_low_precision(reason="bf16 partial out"):
                        nc.vector.tensor_copy(ost[:], pw[:])
                    nc.sync.dma_start(
                        d_part[tsl, 512 * j:512 * (j + 1)], ost[:]
                    )

            def norm_phase1(po):
                # evacuate o' from PSUM right away so the accumulator bank
                # frees for the next group's PV
                oc = ocp.tile([65, 512], F32, tag="oc", name="oc")
                nc.vector.tensor_copy(oc[:], po[0:65, :])
                return oc

            dn2_t = small.tile([33, 512], F32, tag="dn2", name="dn2t", bufs=1)
            nc.vector.memset(dn2_t[:], 1.0)
            rc_t = rcp.tile([33, 512], F32, tag="rc", name="rct", bufs=1)

            def recip_pair(ocA, ocB):
                # both denominator rows packed at partitions 0/32 of one
                # tile; a single reciprocal (cost tracks free-dim) serves
                # both halves
                nc.vector.tensor_copy(dn2_t[0:1, :], ocA[64:65, :])
                nc.vector.tensor_copy(dn2_t[32:33, :], ocB[64:65, :])
                nc.vector.reciprocal(rc_t[:], dn2_t[:])
                dnrA = small.tile([1, 512], F32R, tag="dnrA", name="dnrA", bufs=2)
                nc.vector.tensor_copy(dnrA[0:1, :], rc_t[0:1, :])
                dnrB = small.tile([1, 512], F32R, tag="dnrB", name="dnrB", bufs=2)
                nc.vector.tensor_copy(dnrB[0:1, :], rc_t[32:33, :])
                return (dnrA, dnrB)

            def make_norm(p, qsl, i, oc, dnr):
                # PE-broadcast the reciprocal row, multiply o'^T from PSUM
                def norm():
                    r0 = 64 * i
                    pb = ps_misc.tile([128, 512], F32, tag="ps", name="pb")
                    nc.tensor.matmul(pb[0:64, :], ones_sb[0:1, 0:64],
                                     dnr[i][0:1, :])
                    with nc.allow_low_precision(reason="bf16 normalized out"):
                        nc.vector.tensor_mul(
                            oT_sb[p][r0:r0 + 64, qsl], oc[0:64, :], pb[0:64, :]
                        )
                return norm

            pending_norms = []
            pending_norms2 = []

            for n in range(NQT):
                if n + 1 < NQT:
                    next_qchs = load_chunk(d_qt, n + 1, qinch)
                qsl = slice(512 * n, 512 * (n + 1))
                for p in range(NP):
                    if n == 0 and p > 0:
                        # the f-tiles this pair's scores need, emitted
                        # before the pair's PSUM accumulators so the shared
                        # ring stays acyclic
                        for c in range(NQT):
                            proj_piece(wk_sb, kchs[c], bk_sb, kT_sb, c, p)
                        proj_piece(wq_sb, qchs0, bq_sb, qT_sb, 0, p)
                    poA = ps_pv.tile([128, 512], F32, tag="po")
                    poB = ps_pv.tile([128, 512], F32, tag="po")

                    def sc_emit(m, p=p, qsl=qsl):
                        scp = ps_sc.tile([128, 1024], F32, tag="sc")
                        ksl = slice(128 * m, 128 * (m + 1))
                        nc.tensor.matmul(
                            scp[:, 0:512], kT_sb[p][0:64, ksl], qT_sb[p][0:64, qsl],
                            tile_position=(0, 0),
                        )
                        nc.tensor.matmul(
                            scp[:, 512:1024], kT_sb[p][64:128, ksl],
                            qT_sb[p][64:128, qsl], tile_position=(64, 0),
                        )
                        ex = exch.tile([128, 1024], BF16, tag="ex")
                        nc.scalar.activation(ex[:], scp[:], EXP, scale=SCALE)
                        return ex

                    exs = {0: sc_emit(0), 1: sc_emit(1)}
                    for m in range(KT):
                        if m + 2 < KT:
                            exs[m + 2] = sc_emit(m + 2)
                        ex = exs.pop(m)
                        nc.tensor.matmul(
                            poA[0:65, :], v_sb[m][:, 130 * p:130 * p + 65],
                            ex[:, 0:512], start=(m == 0), stop=(m == KT - 1),
                        )
                        nc.tensor.matmul(
                            poB[0:65, :], v_sb[m][:, 130 * p + 65:130 * p + 130],
                            ex[:, 512:1024], start=(m == 0), stop=(m == KT - 1),
                        )
                        # previous group's normalizes land just after this
                        # group's pipeline is rolling; Wo units fill PE slack
                        if m == 0 and len(pending_norms) >= 2:
                            poA_, pA_, qslA_, iA_ = pending_norms.pop(0)
                            poB_, pB_, qslB_, iB_ = pending_norms.pop(0)
                            ocA = norm_phase1(poA_)
                            ocB = norm_phase1(poB_)
                            dnr = recip_pair(ocA, ocB)
                            pending_norms2.append(
                                make_norm(pA_, qslA_, iA_, ocA, dnr))
                            pending_norms2.append(
                                make_norm(pB_, qslB_, iB_, ocB, dnr))
                        if m in (4, 6) and pending_norms2:
                            pending_norms2.pop(0)()
                        if m in (7, 10, 13):
                            emit_wo(1)
                    if n + 1 < NQT:
                        # next chunk's q projection, one feature tile per group
                        proj_piece(wq_sb, next_qchs, bq_sb, qT_sb, n + 1, p)
                    pending_norms.append((poA, p, qsl, 0))
                    pending_norms.append((poB, p, qsl, 1))
                    if n == NQT - 1:
                        # final chunk: run norms eagerly so the tail is short
                        while pending_norms2:
                            pending_norms2.pop(0)()
                        while pending_norms:
                            poA_, pA_, qslA_, iA_ = pending_norms.pop(0)
                            poB_, pB_, qslB_, iB_ = pending_norms.pop(0)
                            ocA = norm_phase1(poA_)
                            ocB = norm_phase1(poB_)
                            dnr = recip_pair(ocA, ocB)
                            make_norm(pA_, qslA_, iA_, ocA, dnr)()
                            make_norm(pB_, qslB_, iB_, ocB, dnr)()
                        emit_wo(2)
                # queue this chunk's Wo pieces (interleaved into later groups)
                for t in range(4):
                    for j in range(2):
                        wo_pending.append((4 * n + t, j))
            while pending_norms:
                poA_, pA_, qslA_, iA_ = pending_norms.pop(0)
                poB_, pB_, qslB_, iB_ = pending_norms.pop(0)
                ocA = norm_phase1(poA_)
                ocB = norm_phase1(poB_)
                dnr = recip_pair(ocA, ocB)
                make_norm(pA_, qslA_, iA_, ocA, dnr)()
                make_norm(pB_, qslB_, iB_, ocB, dnr)()
            for nm in pending_norms2:
                nm()
            emit_wo(len(wo_pending))

    _split_multiwaits(nc)
    return nc


_PROGRAM = None


def _get_program():
    global _PROGRAM
    if _PROGRAM is None:
        _PROGRAM = build_program()
    return _PROGRAM


def make_in_maps(Q, K, V, Wq, bq, Wk, bk, Wv, bv, Wo, bo):
    import ml_dtypes
    bf = lambda x: np.asarray(x, dtype=np.float32).astype(ml_dtypes.bfloat16)
    f32 = lambda x: np.asarray(x, dtype=np.float32)
    Q, K, V = bf(Q), bf(K), bf(V)
    Wq, Wk, Wv, Wo = bf(Wq), bf(Wk), bf(Wv), bf(Wo)
    bq, bk, bv = f32(bq), f32(bk), f32(bv)
    ones = np.ones((1, 128), np.float32)
    in_maps = []
    for c in range(N_CORES):
        b, hh = c // 2, c % 2
        fs = slice(F * hh, F * (hh + 1))
        in_maps.append({
            "qt": np.ascontiguousarray(Q[b].T),
            "kt": np.ascontiguousarray(K[b].T),
            "vt": np.ascontiguousarray(V[b].T),
            "wq": np.ascontiguousarray(Wq[:, fs]),
            "wk": np.ascontiguousarray(Wk[:, fs]),
            "wv": np.ascontiguousarray(Wv[:, fs]),
            "wo": np.ascontiguousarray(Wo[fs, :]),
            "bq": np.ascontiguousarray(bq[fs]),
            "bk": np.ascontiguousarray(bk[fs]),
            "bv": np.ascontiguousarray(bv[fs]),
            "ones": ones,
        })
    return in_maps


def kernel(Q, K, V, Wq, bq, Wk, bk, Wv, bv, Wo, bo, _trace=False, _trace_kwargs=None):
    nc = _get_program()
    in_maps = make_in_maps(Q, K, V, Wq, bq, Wk, bk, Wv, bv, Wo, bo)
    res = run_bass_kernel_spmd(
        nc, in_maps, core_ids=list(range(N_CORES)),
        trace=_trace, **(_trace_kwargs or {}),
    )
    parts = [np.asarray(r["part"], dtype=np.float32) for r in res.results]
    out = np.stack([parts[2 * b] + parts[2 * b + 1] for b in range(B)])
    out += np.asarray(bo, dtype=np.float32)[None, None, :]
    if _trace:
        return out, res
    return out
